# revision 2
# baseline (speedup 1.0000x reference)
"""Conv3d(16->64, k=3, VALID) + sigmoid(tanh(conv*scale)*bias), B=8 sharded
over 8 NeuronCores (one batch element per core).

v5 "S2" scheme (~2.5x faster than v4's bf16 M=64 4-pass scheme): fp8-e4m3 DoubleRow matmuls (K up to 256 logical rows),
plane-PAIR psum tiles (partitions = (plane parity, co) -> M=128), and
stride-2 input windows so each pair of output planes needs only THREE
full-plane rhs streams:

  - W_q windows (q even): [128 part, 2 ktile, 3968] fp8; logical row
    (delta, kh, kw, ci) with (kh,kw) != (2,2): content x[q+delta] shifted
    kh*S+kw. 16 combos x 16 ci = 256 logical rows.
  - stream A = W_d:   g0 -> plane d taps kd=delta (16 taps, K=256);
                      g1 -> plane d+1 taps kd=0 via delta=1 rows (K=128)
  - stream B = W_d+2: g0 -> plane d kd=2 via delta=0 (K=128);
                      g1 -> plane d+1 kd=delta+1 (16 taps, K=256)
  - stream C = V_d:   [32 part, 2, 3968], rows (gamma, ci), content
    x[d+gamma] shifted 2S+2: the excluded (kh,kw)=(2,2) taps for both
    planes (g0: kd=gamma<=2, g1: kd=gamma-1>=0).

  Per pair: 3 streams x 2 psum half-tiles = ~11.9k PE columns (vs 31.7k
  at M=64 bf16 in v4). Epilogue identical to v4: single ACT Tanh pass
  (sigmoid(b*tanh(a)) ~= 0.5 + beta*tanh(gamma*a)), DVE affine, packed
  bf16 stores; host upcasts bf16 -> fp32.
"""

import sys

sys.path.insert(0, "/opt/trn_rl_repo")

import numpy as np
import ml_dtypes

import concourse.bass as bass
import concourse.mybir as mybir
from concourse import tile
from concourse.bass_utils import run_bass_kernel_spmd

# ---- problem constants (hardcoded per spec) ----
B = 8
CIN = 16
COUT = 64
K = 3
S = 64  # input spatial
SO = S - K + 1  # 62 output spatial
PLANE = S * S  # 4096
NCORES = 8
OPLANE = SO * SO  # 3844 packed output plane
OHALF = 32 * SO  # 1984 cols in the h<32 half

WF = 3968  # window free size (= max rhs col 61*64+63 + 1)
NW = 32  # stride-2 windows W_q, q = 0,2,...,62
NPAIRS = 31
KHKW8 = [(kh, kw) for kh in range(K) for kw in range(K) if (kh, kw) != (2, 2)]

F8 = mybir.dt.float8e4
BF16 = mybir.dt.bfloat16
FP32 = mybir.dt.float32
NP8 = ml_dtypes.float8_e4m3
DR = mybir.MatmulPerfMode.DoubleRow


def split_multiwaits(nc):
    """walrus in this toolchain rejects instructions carrying more than one
    sync-wait. Rewrite every multi-wait instruction into (n-1) single-wait
    nops on the same engine queue followed by the instruction with the last
    wait — identical semantics since each engine queue executes serially."""
    for func in nc.m.functions:
        for block in func.blocks:
            insts = block.instructions
            if not any(
                i.sync_info is not None and len(i.sync_info.on_wait or ()) > 1
                for i in insts
            ):
                continue
            newlist = []
            for inst in insts:
                si = inst.sync_info
                if si is not None and si.on_wait and len(si.on_wait) > 1:
                    waits = list(si.on_wait)
                    for w in waits[:-1]:
                        nop = mybir.InstNoOp(
                            name=nc.get_next_instruction_name(),
                            sync_info=mybir.SyncInfo(on_wait=[w], on_update=[]),
                            bass_nofuse=True,
                            engine=inst.engine,
                        )
                        newlist.append(nop)
                    si.on_wait = waits[-1:]
                newlist.append(inst)
            insts[:] = newlist


class PatchedTileContext(tile.TileContext):
    def __exit__(self, exc_type, exc_value, traceback):
        ret = super().__exit__(exc_type, exc_value, traceback)
        if exc_type is None:
            split_multiwaits(self.nc)
        return ret


def build_nc(npairs=NPAIRS, repeat=1, hwloop=False, ablate=(), qa=4):
    nc = bass.Bass(trn_type="TRN2")
    xw = nc.dram_tensor("xw", [128, NW, 2, WF], F8, kind="ExternalInput")
    xv = nc.dram_tensor("xv", [32, NPAIRS, 2, WF], F8, kind="ExternalInput")
    la = nc.dram_tensor("la", [128, 2 * 128], F8, kind="ExternalInput")
    lb = nc.dram_tensor("lb", [128, 2 * 128], F8, kind="ExternalInput")
    lc = nc.dram_tensor("lc", [32, 2 * 128], F8, kind="ExternalInput")
    gvec = nc.dram_tensor("gvec", [2 * COUT, 1], FP32, kind="ExternalInput")
    bvec = nc.dram_tensor("bvec", [2 * COUT, 1], FP32, kind="ExternalInput")
    out = nc.dram_tensor(
        "out", [2 * COUT, NPAIRS * OPLANE], BF16, kind="ExternalOutput"
    )

    AF = mybir.ActivationFunctionType

    with PatchedTileContext(nc) as tc:
        with (
            tc.tile_pool(name="const", bufs=1) as cpool,
            tc.tile_pool(name="win", bufs=4) as wpool,
            tc.tile_pool(name="vwin", bufs=3) as vpool,
            tc.tile_pool(name="eptmp", bufs=3) as epool,
            tc.tile_pool(name="outp", bufs=3) as opool,
            tc.tile_pool(name="psum", bufs=2, space="PSUM") as pspool,
        ):
            la_sb = cpool.tile([128, 2, 128], F8)
            lb_sb = cpool.tile([128, 2, 128], F8)
            lc_sb = cpool.tile([32, 2, 128], F8)
            gv_sb = cpool.tile([2 * COUT, 1], FP32)
            bv_sb = cpool.tile([2 * COUT, 1], FP32)
            nc.sync.dma_start(la_sb[:], la[:].rearrange("p (t m) -> p t m", t=2))
            nc.sync.dma_start(lb_sb[:], lb[:].rearrange("p (t m) -> p t m", t=2))
            nc.sync.dma_start(lc_sb[:], lc[:].rearrange("p (t m) -> p t m", t=2))
            nc.sync.dma_start(gv_sb[:], gvec[:])
            nc.sync.dma_start(bv_sb[:], bvec[:])

            def run_rep():
                wtiles = {}

                wl = 64 if "loads" in ablate else WF

                def load_w(qi):
                    t = wpool.tile([128, 2, WF], F8, tag="w", name=f"w{qi}")
                    if qa == 1:
                        nc.sync.dma_start(t[:, :, 0:wl], xw[:, qi, :, 0:wl])
                    elif qa in (2, 3):
                        nc.sync.dma_start(t[:, 0, 0:wl], xw[:, qi, 0, 0:wl])
                        nc.gpsimd.dma_start(t[:, 1, 0:wl], xw[:, qi, 1, 0:wl])
                    else:  # qa 4: sync + scalar
                        nc.sync.dma_start(t[:, 0, 0:wl], xw[:, qi, 0, 0:wl])
                        nc.scalar.dma_start(t[:, 1, 0:wl], xw[:, qi, 1, 0:wl])
                    wtiles[qi] = t

                vtiles = {}

                def load_v(pi):
                    t = vpool.tile([32, 2, WF], F8, tag="v", name=f"v{pi}")
                    nc.gpsimd.dma_start(t[:, :, 0:wl], xv[:, pi, :, 0:wl])
                    vtiles[pi] = t

                load_w(0)
                load_w(1)
                load_w(2)
                load_v(0)
                load_v(1)

                for pi in range(npairs):
                    d = 2 * pi
                    wa = wtiles.pop(pi)
                    wb = wtiles[pi + 1]
                    vc = vtiles.pop(pi)
                    # prefetch two pairs ahead
                    if pi + 3 <= npairs:
                        load_w(pi + 3)
                    if pi + 2 < npairs:
                        load_v(pi + 2)

                    for H in range(2):
                        nh = 32 if H == 0 else 30
                        ps = pspool.tile([128, 32, S], FP32, tag="ps", name="ps")
                        # stream order alternates so consecutive halves/pairs
                        # share lhsT at the boundary (4 ldweights per pair)
                        streams = [
                            (la_sb, wa, "A"),
                            (lb_sb, wb, "B"),
                            (lc_sb, vc, "C"),
                        ]
                        if H == 1:
                            streams = streams[::-1]
                        nreg = 1 if "mm" in ablate else 8
                        for si, (lt, rt, _tag) in enumerate(streams):
                            first, last = si == 0, si == 2
                            for r in range(nreg):
                                rh = min(4, nh - r * 4)
                                ncols = rh * S
                                f0 = (H * 32 + r * 4) * S
                                # start=True zeroes the ENTIRE 2KB psum bank
                                # (512 fp32); regions are half-bank, so only
                                # the first (even-r) instr of the first
                                # stream starts — the odd-r partner
                                # accumulates into the already-zeroed half.
                                nc.tensor.matmul(
                                    ps[:, r * 4 : r * 4 + rh, :],
                                    lt[:],
                                    rt[:, :, f0 : f0 + ncols],
                                    start=first and r % 2 == 0,
                                    stop=last,
                                    perf_mode=DR,
                                )
                        # epilogue: tanh(gamma*a) on the valid [128, nh, 62]
                        # region, then 0.5 + beta*t, packed bf16 stores
                        nact = 62 if "act" in ablate else nh * SO
                        act_in = (
                            ps[:, 0:1, 0:SO] if "act" in ablate
                            else ps[:, 0:nh, 0:SO]
                        )
                        t_sb = epool.tile([128, OHALF], BF16, tag="t", name="t")
                        o_sb = opool.tile([128, OHALF], BF16, tag="o", name="o")
                        nc.scalar.activation(
                            t_sb[:, 0:nact], act_in, AF.Tanh,
                            scale=gv_sb[:],
                        )
                        nc.vector.tensor_scalar(
                            o_sb[:, 0:nact],
                            t_sb[:, 0:nact],
                            bv_sb[:],
                            0.5,
                            mybir.AluOpType.mult,
                            mybir.AluOpType.add,
                        )
                        nst = 64 if "stores" in ablate else nact
                        eng = (nc.scalar, nc.sync)[H] if qa == 4 else nc.sync
                        b0 = pi * OPLANE + H * OHALF
                        eng.dma_start(
                            out[:, b0 : b0 + nst], o_sb[:, 0:nst]
                        )

            if hwloop and repeat > 1:
                # body is iteration-independent; hardware loop keeps the
                # instruction count (and compile time) flat in `repeat`
                with tc.For_i(0, repeat):
                    run_rep()
            else:
                for _ in range(repeat):
                    run_rep()
    return nc


def _sigmoid(z):
    return 1.0 / (1.0 + np.exp(-z))


def prepare_in_maps(x, weight, scale, bias):
    x = np.asarray(x, dtype=np.float32)
    weight = np.asarray(weight, dtype=np.float32)
    scale = np.asarray(scale, dtype=np.float32)
    bias = np.asarray(bias, dtype=np.float32)

    # fold scale into weights, quantize once to fp8
    w_eff = weight * scale.reshape(COUT, 1, 1, 1, 1)  # [co, ci, kd, kh, kw]
    w8 = w_eff.astype(NP8).astype(np.float32)  # [co, ci, kd, kh, kw]

    # lhsT matrices [rows, ktile, (g, co)]
    la = np.zeros((128, 2, 128), dtype=np.float32)
    lb = np.zeros((128, 2, 128), dtype=np.float32)
    lc = np.zeros((32, 2, 128), dtype=np.float32)
    for c in range(16):
        delta, e = divmod(c, 8)
        kh, kw = KHKW8[e]
        for j in range(8):
            for t in range(2):
                ci = j * 2 + t
                p = c * 8 + j
                # wT[ci, tap] -> col co
                la[p, t, 0:64] = w8[:, ci, delta, kh, kw]
                if delta == 1:
                    la[p, t, 64:128] = w8[:, ci, 0, kh, kw]
                if delta == 0:
                    lb[p, t, 0:64] = w8[:, ci, 2, kh, kw]
                lb[p, t, 64:128] = w8[:, ci, delta + 1, kh, kw]
    for g in range(4):
        for j in range(8):
            for t in range(2):
                ci = j * 2 + t
                p = g * 8 + j
                if g <= 2:
                    lc[p, t, 0:64] = w8[:, ci, g, 2, 2]
                if g >= 1:
                    lc[p, t, 64:128] = w8[:, ci, g - 1, 2, 2]
    la8 = la.reshape(128, 256).astype(NP8)
    lb8 = lb.reshape(128, 256).astype(NP8)
    lc8 = lc.reshape(32, 256).astype(NP8)

    # epilogue vectors: out ~= 0.5 + beta * tanh(gamma * a)
    b = bias.reshape(COUT).astype(np.float64)
    beta = _sigmoid(b) - 0.5
    gamma = np.where(np.abs(b) < 1e-3, 1.0 + b * b / 12.0, b / (4.0 * beta))
    gv = np.tile(gamma.astype(np.float32), 2).reshape(2 * COUT, 1)
    bv = np.tile(beta.astype(np.float32), 2).reshape(2 * COUT, 1)

    # fp8 cast + padded flat volume per core: [16, 66*PLANE]
    x8 = x.astype(NP8)
    xpad = np.zeros((B, CIN, 66 * PLANE), dtype=NP8)
    xpad[:, :, 0 : S * PLANE] = x8.reshape(B, CIN, S * PLANE)

    st = np.lib.stride_tricks.as_strided

    # windows xw [B, 128, NW, 2, WF]: row (c*8+j, t) = x[2q+delta] shifted
    xw = np.empty((B, 128, NW, 2, WF), dtype=NP8)
    sb, sc, se = xpad.strides
    for c in range(16):
        delta, e = divmod(c, 8)
        kh, kw = KHKW8[e]
        off = delta * PLANE + kh * S + kw
        view = st(
            xpad[:, :, off:],
            shape=(B, CIN, NW, WF),
            strides=(sb, sc, 2 * PLANE * se, se),
        )
        for j in range(8):
            for t in range(2):
                xw[:, c * 8 + j, :, t, :] = view[:, j * 2 + t]

    # leftover xv [B, 32, NPAIRS, 2, WF]: row (g*8+j, t) = x[d+g] + 2S+2
    xv = np.empty((B, 32, NPAIRS, 2, WF), dtype=NP8)
    for g in range(4):
        off = g * PLANE + 2 * S + 2
        view = st(
            xpad[:, :, off:],
            shape=(B, CIN, NPAIRS, WF),
            strides=(sb, sc, 2 * PLANE * se, se),
        )
        for j in range(8):
            for t in range(2):
                xv[:, g * 8 + j, :, t, :] = view[:, j * 2 + t]

    return [
        {
            "xw": xw[c],
            "xv": xv[c],
            "la": la8,
            "lb": lb8,
            "lc": lc8,
            "gvec": gv,
            "bvec": bv,
        }
        for c in range(NCORES)
    ]


_NC_CACHE = None
LAST_RESULT = None


def kernel(x, weight, scale, bias):
    global _NC_CACHE, LAST_RESULT
    in_maps = prepare_in_maps(x, weight, scale, bias)

    if _NC_CACHE is None:
        _NC_CACHE = build_nc()
    nc = _NC_CACHE

    res = run_bass_kernel_spmd(nc, in_maps, list(range(NCORES)))
    LAST_RESULT = res

    out = np.empty((B, COUT, SO, SO, SO), dtype=np.float32)
    for c in range(NCORES):
        r = res.results[c]["out"].astype(np.float32)
        r = r.reshape(2, COUT, NPAIRS, OPLANE)  # [j, co, pi, plane]
        out[c] = r.transpose(1, 2, 0, 3).reshape(COUT, SO, SO, SO)
    return out
